# revision 33
# baseline (speedup 1.0000x reference)
"""Binary 3D dilation (star/6-connected structuring element) on 8 TRN2 cores.

out = (conv3d(x, star_kernel, pad=1) > 0)  for x in {0,1}^(2,1,256,256,256)

BIT-PACKED formulation: the volume is binary, so pack voxels into
uint32 words along W (host-side, free).  Words OVERLAP by 2 bits: word
j holds voxels w = 30j-1 .. 30j+30 (30 payload bits + 1 halo bit each
end), so the W-stencil never crosses a word boundary and the dilation
is a pure bitwise OR of 7 terms per packed word:

    out = C | (C<<1) | (C>>1)       # W-stencil (bits 0/31 are garbage,
        | X[d-1] | X[d+1]           #   discarded by the host unpack)
        | X[h-1] | X[h+1]           # D- and H-stencils

scalar_tensor_tensor fuses (shift | OR) in one DVE instruction, so a
block is 6 instructions (the provable minimum: 7 terms, binary ops)
over ~1/28 the data of the float formulation.  DMA traffic drops ~6x
vs the fp8 baseline.  (The no-overlap variant needs cross-word carry
ops whose partial-word APs are 4D — the walrus verifier limits
ScalarTensorTensor to 3D APs.  Bitwise ALU ops only exist on DVE, so
all compute is on DVE; Pool/ACT/PE have nothing to contribute.)

The 6 ops form a balanced tree (depth 3, three independent leading
ops) rather than a serial in-place chain — the tile scheduler overlaps
independent ops, hiding the ~140ns/op semaphore round-trip:
    Y  = (C<<1) | C      T2 = (C>>1) | d-      T3 = d+ | h-
    Y |= h+              T2 |= T3
    Y |= T2   -> store

Sharding: core k -> batch k//4, D-quarter k%4 (64 output planes/core).
Partition layout: p = hb*8 + dq with hb in [0,16) blocks of 16 H-rows
and dq in [0,8) blocks of 8 D-planes.  Each partition holds its block
plus a 1-plane / 1-row halo on each side (host-duplicated, zero at
volume boundaries): X[p] = [10 planes, 18 rows, 9 words] uint32.
All stencil axes are then free-dim offsets within the partition.

Pipeline: plane-blocks (2, 4, 2) per partition; loads are chunked so
each block's first 5 ops need only the chunk before its last plane
(the d+ OR is emitted last); stores fire per block on SP/ACT.

Measured: 23.2us vs 81.5us for the fp8/matmul baseline (3.5x).  ~13us
of that is fixed harness overhead (NEFF startup, instruction loads,
barriers, and a ~250-instruction full semaphore-file sweep in the
epilogue) that a trivial 1-op kernel also pays; the work portion is
~10us: ~8.3us DVE exec (vs 7.2us theoretical at 32 bits/lane/cycle)
+ load/store edges.  DMA totals 1.42 MB/core at ~205 GB/s effective
(8 cores share chip HBM bandwidth).
"""

import sys

import numpy as np

if "/opt/trn_rl_repo" not in sys.path:
    sys.path.insert(0, "/opt/trn_rl_repo")

B = 2
D_TOT = 256
H = 256
W = 256
PAY = 30                  # payload bits per overlap-packed word
NW = 9                    # words per row: ceil(256/30) with 1-bit halos
N_CORES = 8
D_SHARDS = 4              # D split per batch entry
D_OUT = D_TOT // D_SHARDS           # 64 output planes per core

N_HB = 16                 # H blocks per core (partition dim)
RPP = H // N_HB           # 16 output rows per partition
N_DB = 8                  # D blocks per core (partition dim)
DPP = D_OUT // N_DB       # 8 output planes per partition
DL = DPP + 2              # input planes per partition (with halo)
RL = RPP + 2              # input rows per partition (with halo)

# 6-connected "star" structuring element mask (D,H,W offsets from center)
_STAR = np.zeros((3, 3, 3), bool)
_STAR[1, 1, 1] = _STAR[0, 1, 1] = _STAR[2, 1, 1] = True
_STAR[1, 0, 1] = _STAR[1, 2, 1] = True
_STAR[1, 1, 0] = _STAR[1, 1, 2] = True

# extra kwargs for run_bass_kernel_spmd (test.py sets trace=True here)
RUN_KWARGS: dict = {}
LAST_RESULTS = None


def build_nc(blocks=(2, 4, 2)):
    """Per-core Bass program (identical on all cores).

    blocks: output planes per compute block (pipeline granularity).
    Small first block -> compute starts after a small load chunk; big
    middle blocks -> low per-instruction overhead; small last block ->
    short store tail.  All compute on DVE (the only engine with 32-bit
    bitwise ALU support).
    """
    import concourse.bass as bass
    import concourse.mybir as mybir
    import concourse.tile as tile

    u32 = mybir.dt.uint32
    OR = mybir.AluOpType.bitwise_or
    SHL = mybir.AluOpType.logical_shift_left
    SHR = mybir.AluOpType.logical_shift_right

    assert sum(blocks) == DPP

    nc = bass.Bass()
    x = nc.declare_dram_parameter("x", [128, DL, RL, NW], u32, isOutput=False)
    y = nc.declare_dram_parameter("y", [128, DPP, RPP, NW], u32, isOutput=True)

    with tile.TileContext(nc) as tc:
        with (
            tc.tile_pool(name="consts", bufs=1) as cpool,
            tc.tile_pool(name="data", bufs=1) as dpool,
            tc.tile_pool(name="work", bufs=2) as wpool,
        ):
            s1 = cpool.tile([128, 1], u32, tag="s1")
            nc.vector.memset(s1[:], 1)

            X = dpool.tile([128, DL, RL, NW], u32, tag="x")
            Y = dpool.tile([128, DPP, RPP, NW], u32, tag="y")

            # ---- input loads: chunked over planes, alternate SP/ACT ----
            # block k (out planes [a0, a0+bpl)) needs input dl [a0, a0+bpl+2),
            # but its first 5 ops only need dl [a0, a0+bpl+1) -- the d+ OR
            # (emitted last) is the only reader of the final plane.  Chunk
            # boundaries at a0+bpl+1 let each block start one plane early.
            load_engines = [nc.sync, nc.scalar]
            chunks = []
            hi0 = 0
            a0 = 0
            for bpl in blocks:
                need = a0 + bpl + 1
                if need > hi0:
                    chunks.append((hi0, need))
                    hi0 = need
                a0 += bpl
            if hi0 < DL:
                chunks.append((hi0, DL))
            for i, (lo, hi) in enumerate(chunks):
                # chunks after the first re-load one plane of the previous
                # chunk: the WAW overlap makes the tile framework chain the
                # DMAs, so chunk0 (which gates compute start) gets the full
                # DMA-engine bandwidth instead of sharing it with later
                # chunks whose data isn't needed for several microseconds
                if i > 0:
                    lo -= 1
                eng = load_engines[i % 2]
                eng.dma_start(out=X[:, lo:hi], in_=x[:, lo:hi])

            # ---- compute: all on DVE ----------------------------------
            # Balanced OR tree per block (depth 3, three independent leading
            # ops) instead of a serial 6-op chain: the list scheduler can
            # overlap independent ops, hiding the ~140ns/op semaphore
            # round-trip of a serial in-place chain.
            #   Y  = (C<<1) | C          T2 = (C>>1) | d-      T3 = d+ | h-
            #   Y |= h+                  T2 |= T3
            #   Y |= T2   -> store
            a0 = 0
            for k, bpl in enumerate(blocks):
                C = X[:, a0 + 1 : a0 + 1 + bpl, 1 : RPP + 1, :]
                T = Y[:, a0 : a0 + bpl, :, :]
                T2 = wpool.tile([128, bpl, RPP, NW], u32, tag=f"t2_{bpl}")
                T3 = wpool.tile([128, bpl, RPP, NW], u32, tag=f"t3_{bpl}")
                dm = X[:, a0 : a0 + bpl, 1 : RPP + 1, :]
                dp = X[:, a0 + 2 : a0 + 2 + bpl, 1 : RPP + 1, :]
                hm = X[:, a0 + 1 : a0 + 1 + bpl, 0:RPP, :]
                hp = X[:, a0 + 1 : a0 + 1 + bpl, 2 : RPP + 2, :]
                nc.vector.scalar_tensor_tensor(
                    out=T, in0=C, scalar=s1[:], in1=C, op0=SHL, op1=OR
                )
                nc.vector.scalar_tensor_tensor(
                    out=T2[:], in0=C, scalar=s1[:], in1=dm, op0=SHR, op1=OR
                )
                nc.vector.tensor_tensor(out=T3[:], in0=dp, in1=hm, op=OR)
                nc.vector.tensor_tensor(out=T, in0=hp, in1=T, op=OR)
                nc.vector.tensor_tensor(out=T2[:], in0=T3[:], in1=T2[:], op=OR)
                store_eng = load_engines[(k + 1) % 2]
                if k < len(blocks) - 1:
                    nc.vector.tensor_tensor(out=T, in0=T2[:], in1=T, op=OR)
                    store_eng.dma_start(
                        out=y[:, a0 : a0 + bpl], in_=Y[:, a0 : a0 + bpl]
                    )
                else:
                    # last block: final OR + store per plane, so the first
                    # plane's store overlaps the remaining planes' ORs and
                    # only one small store sits on the critical tail
                    for j in range(bpl):
                        nc.vector.tensor_tensor(
                            out=Y[:, a0 + j : a0 + j + 1, :, :],
                            in0=T2[:, j : j + 1, :, :],
                            in1=Y[:, a0 + j : a0 + j + 1, :, :],
                            op=OR,
                        )
                        load_engines[(k + 1 + j) % 2].dma_start(
                            out=y[:, a0 + j : a0 + j + 1],
                            in_=Y[:, a0 + j : a0 + j + 1],
                        )
                a0 += bpl

    import bass_rust as _bass_rust

    _bass_rust.generate_event_semaphores(nc)
    return nc


_NC_CACHE = None


def pack_volume(vol4: np.ndarray) -> np.ndarray:
    """(B, D, H, W) 0/1 float -> padded packed bits [B, D+2, H+2, NW] u32.

    Overlap packing: bit i of word j = voxel w = PAY*j - 1 + i, so each
    word carries its own 1-voxel W-halo and shifts never cross words.
    """
    bits = (vol4 != 0).astype(np.uint8)
    bb = np.zeros((B, D_TOT, H, PAY * NW + 2), np.uint8)  # w in [-1, 271)
    bb[..., 1 : W + 1] = bits
    win = np.lib.stride_tricks.sliding_window_view(bb, 32, axis=-1)
    win = win[..., :: PAY, :]                            # [B,D,H,NW,32]
    pk = np.packbits(win, axis=-1, bitorder="little")    # [B,D,H,NW,4] u8
    p32 = np.ascontiguousarray(pk).view(np.uint32)[..., 0]
    pad = np.zeros((B, D_TOT + 2, H + 2, NW), np.uint32)
    pad[:, 1:-1, 1:-1, :] = p32
    return pad


_DI = (DPP * np.arange(N_DB))[:, None] + np.arange(DL)   # [N_DB, DL]
_HI = (RPP * np.arange(N_HB))[:, None] + np.arange(RL)   # [N_HB, RL]


def core_input(ppad: np.ndarray, core: int) -> dict:
    """Per-core in_map from the padded packed volume."""
    b, q = divmod(core, D_SHARDS)
    sub = ppad[b, q * D_OUT : q * D_OUT + D_OUT + 2]     # [66, 258, NW]
    xd = sub[_DI]                                        # [N_DB, DL, 258, NW]
    xh = xd[:, :, _HI]                                   # [N_DB, DL, N_HB, RL, NW]
    X = np.ascontiguousarray(xh.transpose(2, 0, 1, 3, 4)).reshape(128, DL, RL, NW)
    return {"x": X}


def core_output(yh: np.ndarray) -> np.ndarray:
    """[128, DPP, RPP, NW] u32 -> (D_OUT, H, W) float32."""
    r = yh.reshape(N_HB, N_DB, DPP, RPP, NW).transpose(1, 2, 0, 3, 4)
    r = np.ascontiguousarray(r).view(np.uint8).reshape(D_OUT, H, NW * 4)
    bits = np.unpackbits(r, axis=-1, bitorder="little")  # [D_OUT, H, NW*32]
    bits = bits.reshape(D_OUT, H, NW, 32)[..., 1:31]     # drop halo bits
    return bits.reshape(D_OUT, H, NW * PAY)[..., :W].astype(np.float32)


def _np_dilate(vol: np.ndarray, ker: np.ndarray) -> np.ndarray:
    """Generic numpy fallback: conv3d(pad=1) > 0 for an arbitrary 3x3x3
    kernel (matches the reference exactly, including negative weights)."""
    b, ch, dd, hh, ww = vol.shape
    pad = np.pad(vol, ((0, 0), (0, 0), (1, 1), (1, 1), (1, 1)))
    kv = ker.reshape(3, 3, 3).astype(np.float64)
    s = np.zeros(vol.shape, np.float64)
    for i in range(3):
        for j in range(3):
            for k in range(3):
                if kv[i, j, k] != 0.0:
                    s += kv[i, j, k] * pad[:, :, i : i + dd, j : j + hh, k : k + ww]
    return (s > 0).astype(vol.dtype)


def kernel(binary_volume=None, kernel=None, **_unused):
    global _NC_CACHE, LAST_RESULTS
    vol = np.asarray(binary_volume)
    ker = np.asarray(kernel, dtype=np.float32)
    kv = ker.reshape(3, 3, 3)
    volf = np.ascontiguousarray(vol, dtype=np.float32)
    if (
        vol.shape != (B, 1, D_TOT, H, W)
        or not np.array_equal(kv != 0, _STAR)
        or not (kv[_STAR] > 0).all()
        or not ((volf == 0.0) | (volf == 1.0)).all()
    ):
        return _np_dilate(volf, ker).astype(vol.dtype)

    from concourse.bass_utils import run_bass_kernel_spmd

    ppad = pack_volume(volf.reshape(B, D_TOT, H, W))
    in_maps = [core_input(ppad, core) for core in range(N_CORES)]

    if _NC_CACHE is None:
        _NC_CACHE = build_nc()
    res = run_bass_kernel_spmd(_NC_CACHE, in_maps, list(range(N_CORES)), **RUN_KWARGS)
    LAST_RESULTS = res

    full = np.empty((B, 1, D_TOT, H, W), np.float32)
    for core in range(N_CORES):
        b, q = divmod(core, D_SHARDS)
        full[b, 0, q * D_OUT : (q + 1) * D_OUT] = core_output(res.results[core]["y"])
    return full


# revision 34
# speedup vs baseline: 1.3255x; 1.3255x over previous
"""Binary 3D dilation (star/6-connected structuring element) on 8 TRN2 cores.

out = (conv3d(x, star_kernel, pad=1) > 0)  for x in {0,1}^(2,1,256,256,256)

BIT-PACKED formulation: the volume is binary, so pack voxels into
uint32 words along W (host-side, free).  Words OVERLAP by 2 bits: word
j holds voxels w = 30j-1 .. 30j+30 (30 payload bits + 1 halo bit each
end), so the W-stencil never crosses a word boundary and the dilation
is a pure bitwise OR of 7 terms per packed word:

    out = C | (C<<1) | (C>>1)       # W-stencil (bits 0/31 are garbage,
        | X[d-1] | X[d+1]           #   discarded by the host unpack)
        | X[h-1] | X[h+1]           # D- and H-stencils

scalar_tensor_tensor fuses (shift | OR) in one DVE instruction, so a
block is 6 instructions (the provable minimum: 7 terms, binary ops)
over ~1/28 the data of the float formulation.  DMA traffic drops ~6x
vs the fp8 baseline.  (The no-overlap variant needs cross-word carry
ops whose partial-word APs are 4D — the walrus verifier limits
ScalarTensorTensor to 3D APs.  Bitwise ALU ops only exist on DVE, so
all compute is on DVE; Pool/ACT/PE have nothing to contribute.)

The 6 ops form a balanced tree (depth 3, three independent leading
ops) rather than a serial in-place chain — the tile scheduler overlaps
independent ops, hiding the ~140ns/op semaphore round-trip:
    Y  = (C<<1) | C      T2 = (C>>1) | d-      T3 = d+ | h-
    Y |= h+              T2 |= T3
    Y |= T2   -> store

Sharding: core k -> batch k//4, D-quarter k%4 (64 output planes/core).
Partition layout: p = hb*8 + dq with hb in [0,16) blocks of 16 H-rows
and dq in [0,8) blocks of 8 D-planes.  Each partition holds its block
plus a 1-plane / 1-row halo on each side (host-duplicated, zero at
volume boundaries): X[p] = [10 planes, 18 rows, 9 words] uint32.
All stencil axes are then free-dim offsets within the partition.

Pipeline: plane-blocks (2, 4, 2) per partition; loads are chunked so
each block's first 5 ops need only the chunk before its last plane
(the d+ OR is emitted last); stores fire per block on SP/ACT.

Measured: 23.2us vs 81.5us for the fp8/matmul baseline (3.5x).  ~13us
of that is fixed harness overhead (NEFF startup, instruction loads,
barriers, and a ~250-instruction full semaphore-file sweep in the
epilogue) that a trivial 1-op kernel also pays; the work portion is
~10us: ~8.3us DVE exec (vs 7.2us theoretical at 32 bits/lane/cycle)
+ load/store edges.  DMA totals 1.42 MB/core at ~205 GB/s effective
(8 cores share chip HBM bandwidth).
"""

import sys

import numpy as np

if "/opt/trn_rl_repo" not in sys.path:
    sys.path.insert(0, "/opt/trn_rl_repo")

B = 2
D_TOT = 256
H = 256
W = 256
PAY = 30                  # payload bits per overlap-packed word
NW = 9                    # words per row: ceil(256/30) with 1-bit halos
N_CORES = 8
D_SHARDS = 4              # D split per batch entry
D_OUT = D_TOT // D_SHARDS           # 64 output planes per core

N_HB = 16                 # H blocks per core (partition dim)
RPP = H // N_HB           # 16 output rows per partition
N_DB = 8                  # D blocks per core (partition dim)
DPP = D_OUT // N_DB       # 8 output planes per partition
DL = DPP + 2              # input planes per partition (with halo)
RL = RPP + 2              # input rows per partition (with halo)

# 6-connected "star" structuring element mask (D,H,W offsets from center)
_STAR = np.zeros((3, 3, 3), bool)
_STAR[1, 1, 1] = _STAR[0, 1, 1] = _STAR[2, 1, 1] = True
_STAR[1, 0, 1] = _STAR[1, 2, 1] = True
_STAR[1, 1, 0] = _STAR[1, 1, 2] = True

# extra kwargs for run_bass_kernel_spmd (test.py sets trace=True here)
RUN_KWARGS: dict = {}
LAST_RESULTS = None


def build_nc(blocks=(2, 4, 2)):
    """Per-core Bass program (identical on all cores).

    blocks: output planes per compute block (pipeline granularity).
    Small first block -> compute starts after a small load chunk; big
    middle blocks -> low per-instruction overhead; small last block ->
    short store tail.  All compute on DVE (the only engine with 32-bit
    bitwise ALU support).
    """
    import concourse.bass as bass
    import concourse.mybir as mybir
    import concourse.tile as tile

    u32 = mybir.dt.uint32
    OR = mybir.AluOpType.bitwise_or
    SHL = mybir.AluOpType.logical_shift_left
    SHR = mybir.AluOpType.logical_shift_right

    assert sum(blocks) == DPP

    nc = bass.Bass()
    x = nc.declare_dram_parameter("x", [128, DL, RL, NW], u32, isOutput=False)
    y = nc.declare_dram_parameter("y", [128, DPP, RPP, NW], u32, isOutput=True)

    with tile.TileContext(nc) as tc:
        with (
            tc.tile_pool(name="consts", bufs=1) as cpool,
            tc.tile_pool(name="data", bufs=1) as dpool,
            tc.tile_pool(name="work", bufs=2) as wpool,
        ):
            s1 = cpool.tile([128, 1], u32, tag="s1")
            nc.vector.memset(s1[:], 1)

            X = dpool.tile([128, DL, RL, NW], u32, tag="x")
            Y = dpool.tile([128, DPP, RPP, NW], u32, tag="y")

            # ---- input loads: chunked over planes, alternate SP/ACT ----
            # block k (out planes [a0, a0+bpl)) needs input dl [a0, a0+bpl+2),
            # but its first 5 ops only need dl [a0, a0+bpl+1) -- the d+ OR
            # (emitted last) is the only reader of the final plane.  Chunk
            # boundaries at a0+bpl+1 let each block start one plane early.
            load_engines = [nc.sync, nc.scalar]
            chunks = []
            hi0 = 0
            a0 = 0
            for bpl in blocks:
                need = a0 + bpl + 1
                if need > hi0:
                    chunks.append((hi0, need))
                    hi0 = need
                a0 += bpl
            if hi0 < DL:
                chunks.append((hi0, DL))
            for i, (lo, hi) in enumerate(chunks):
                eng = load_engines[i % 2]
                eng.dma_start(out=X[:, lo:hi], in_=x[:, lo:hi])

            # ---- compute: all on DVE ----------------------------------
            # Balanced OR tree per block (depth 3, three independent leading
            # ops) instead of a serial 6-op chain: the list scheduler can
            # overlap independent ops, hiding the ~140ns/op semaphore
            # round-trip of a serial in-place chain.
            #   Y  = (C<<1) | C          T2 = (C>>1) | d-      T3 = d+ | h-
            #   Y |= h+                  T2 |= T3
            #   Y |= T2   -> store
            a0 = 0
            for k, bpl in enumerate(blocks):
                C = X[:, a0 + 1 : a0 + 1 + bpl, 1 : RPP + 1, :]
                T = Y[:, a0 : a0 + bpl, :, :]
                T2 = wpool.tile([128, bpl, RPP, NW], u32, tag=f"t2_{bpl}")
                T3 = wpool.tile([128, bpl, RPP, NW], u32, tag=f"t3_{bpl}")
                dm = X[:, a0 : a0 + bpl, 1 : RPP + 1, :]
                dp = X[:, a0 + 2 : a0 + 2 + bpl, 1 : RPP + 1, :]
                hm = X[:, a0 + 1 : a0 + 1 + bpl, 0:RPP, :]
                hp = X[:, a0 + 1 : a0 + 1 + bpl, 2 : RPP + 2, :]
                nc.vector.scalar_tensor_tensor(
                    out=T, in0=C, scalar=s1[:], in1=C, op0=SHL, op1=OR
                )
                nc.vector.scalar_tensor_tensor(
                    out=T2[:], in0=C, scalar=s1[:], in1=dm, op0=SHR, op1=OR
                )
                nc.vector.tensor_tensor(out=T3[:], in0=dp, in1=hm, op=OR)
                nc.vector.tensor_tensor(out=T, in0=hp, in1=T, op=OR)
                nc.vector.tensor_tensor(out=T2[:], in0=T3[:], in1=T2[:], op=OR)
                store_eng = load_engines[(k + 1) % 2]
                if k < len(blocks) - 1:
                    nc.vector.tensor_tensor(out=T, in0=T2[:], in1=T, op=OR)
                    store_eng.dma_start(
                        out=y[:, a0 : a0 + bpl], in_=Y[:, a0 : a0 + bpl]
                    )
                else:
                    # last block: final OR + store per plane, so the first
                    # plane's store overlaps the remaining planes' ORs and
                    # only one small store sits on the critical tail
                    for j in range(bpl):
                        nc.vector.tensor_tensor(
                            out=Y[:, a0 + j : a0 + j + 1, :, :],
                            in0=T2[:, j : j + 1, :, :],
                            in1=Y[:, a0 + j : a0 + j + 1, :, :],
                            op=OR,
                        )
                        load_engines[(k + 1 + j) % 2].dma_start(
                            out=y[:, a0 + j : a0 + j + 1],
                            in_=Y[:, a0 + j : a0 + j + 1],
                        )
                a0 += bpl

    import bass_rust as _bass_rust

    _bass_rust.generate_event_semaphores(nc)
    return nc


_NC_CACHE = None


def pack_volume(vol4: np.ndarray) -> np.ndarray:
    """(B, D, H, W) 0/1 float -> padded packed bits [B, D+2, H+2, NW] u32.

    Overlap packing: bit i of word j = voxel w = PAY*j - 1 + i, so each
    word carries its own 1-voxel W-halo and shifts never cross words.
    """
    bits = (vol4 != 0).astype(np.uint8)
    bb = np.zeros((B, D_TOT, H, PAY * NW + 2), np.uint8)  # w in [-1, 271)
    bb[..., 1 : W + 1] = bits
    win = np.lib.stride_tricks.sliding_window_view(bb, 32, axis=-1)
    win = win[..., :: PAY, :]                            # [B,D,H,NW,32]
    pk = np.packbits(win, axis=-1, bitorder="little")    # [B,D,H,NW,4] u8
    p32 = np.ascontiguousarray(pk).view(np.uint32)[..., 0]
    pad = np.zeros((B, D_TOT + 2, H + 2, NW), np.uint32)
    pad[:, 1:-1, 1:-1, :] = p32
    return pad


_DI = (DPP * np.arange(N_DB))[:, None] + np.arange(DL)   # [N_DB, DL]
_HI = (RPP * np.arange(N_HB))[:, None] + np.arange(RL)   # [N_HB, RL]


def core_input(ppad: np.ndarray, core: int) -> dict:
    """Per-core in_map from the padded packed volume."""
    b, q = divmod(core, D_SHARDS)
    sub = ppad[b, q * D_OUT : q * D_OUT + D_OUT + 2]     # [66, 258, NW]
    xd = sub[_DI]                                        # [N_DB, DL, 258, NW]
    xh = xd[:, :, _HI]                                   # [N_DB, DL, N_HB, RL, NW]
    X = np.ascontiguousarray(xh.transpose(2, 0, 1, 3, 4)).reshape(128, DL, RL, NW)
    return {"x": X}


def core_output(yh: np.ndarray) -> np.ndarray:
    """[128, DPP, RPP, NW] u32 -> (D_OUT, H, W) float32."""
    r = yh.reshape(N_HB, N_DB, DPP, RPP, NW).transpose(1, 2, 0, 3, 4)
    r = np.ascontiguousarray(r).view(np.uint8).reshape(D_OUT, H, NW * 4)
    bits = np.unpackbits(r, axis=-1, bitorder="little")  # [D_OUT, H, NW*32]
    bits = bits.reshape(D_OUT, H, NW, 32)[..., 1:31]     # drop halo bits
    return bits.reshape(D_OUT, H, NW * PAY)[..., :W].astype(np.float32)


def _np_dilate(vol: np.ndarray, ker: np.ndarray) -> np.ndarray:
    """Generic numpy fallback: conv3d(pad=1) > 0 for an arbitrary 3x3x3
    kernel (matches the reference exactly, including negative weights)."""
    b, ch, dd, hh, ww = vol.shape
    pad = np.pad(vol, ((0, 0), (0, 0), (1, 1), (1, 1), (1, 1)))
    kv = ker.reshape(3, 3, 3).astype(np.float64)
    s = np.zeros(vol.shape, np.float64)
    for i in range(3):
        for j in range(3):
            for k in range(3):
                if kv[i, j, k] != 0.0:
                    s += kv[i, j, k] * pad[:, :, i : i + dd, j : j + hh, k : k + ww]
    return (s > 0).astype(vol.dtype)


def kernel(binary_volume=None, kernel=None, **_unused):
    global _NC_CACHE, LAST_RESULTS
    vol = np.asarray(binary_volume)
    ker = np.asarray(kernel, dtype=np.float32)
    kv = ker.reshape(3, 3, 3)
    volf = np.ascontiguousarray(vol, dtype=np.float32)
    if (
        vol.shape != (B, 1, D_TOT, H, W)
        or not np.array_equal(kv != 0, _STAR)
        or not (kv[_STAR] > 0).all()
        or not ((volf == 0.0) | (volf == 1.0)).all()
    ):
        return _np_dilate(volf, ker).astype(vol.dtype)

    from concourse.bass_utils import run_bass_kernel_spmd

    ppad = pack_volume(volf.reshape(B, D_TOT, H, W))
    in_maps = [core_input(ppad, core) for core in range(N_CORES)]

    if _NC_CACHE is None:
        _NC_CACHE = build_nc()
    res = run_bass_kernel_spmd(_NC_CACHE, in_maps, list(range(N_CORES)), **RUN_KWARGS)
    LAST_RESULTS = res

    full = np.empty((B, 1, D_TOT, H, W), np.float32)
    for core in range(N_CORES):
        b, q = divmod(core, D_SHARDS)
        full[b, 0, q * D_OUT : (q + 1) * D_OUT] = core_output(res.results[core]["y"])
    return full
